# revision 28
# baseline (speedup 1.0000x reference)
"""Trainium2 Bass kernel for nn_Network_41918880809426 (tree-LSTM message passing).

Structure:
  - B=16384 nodes sharded across 8 cores (2048 each).
  - Phase 1 (parallel, memory-bound): per-layer gate pre-activations
    gates_x = [op|attr] @ W1 + relu([filt|out] @ Wbd + eb) @ W34 + bias,
    computed feature-major by streaming matmuls.
  - Phase 2 (sequential over 24 layers): child gather via indirect DMA from a
    DRAM state table (replicated each layer via AllGather), LSTM cell with
    node-major pointwise ops, PE transposes to bridge layouts.

kernel(**inputs) takes full unsharded inputs, returns full [B,1] output.
"""
import os
import sys

sys.path.insert(0, "/opt/trn_rl_repo")

import numpy as np

import concourse.bass as bass
import concourse.mybir as mybir
import concourse.tile as tile
from concourse import bacc
from concourse.bass_utils import run_bass_kernel_spmd
from concourse.masks import make_identity

F32 = mybir.dt.float32
I32 = mybir.dt.int32
I16 = mybir.dt.int16
AF = mybir.ActivationFunctionType
ALU = mybir.AluOpType

D_OP, D_ATTR, D_FILT, D_OUT, H = 32, 64, 70, 56, 20
DA = D_OP + D_ATTR            # 96  (xa rows)
DB = D_FILT + D_OUT           # 126 (xb rows)
G4 = 4 * H                    # 80 gate rows
SROW = 2 * H                  # 40 floats per state row (h|c)
SPAD = 64                     # padded state row (256B, dma_gather elem size)
ZPAD = 16                     # zero rows appended to each core's shard


def build_program(L, B, n_cores):
    """Build the SPMD bass program. Identical for all cores; all per-core
    variation comes through input tensors."""
    BL = B // n_cores             # local nodes per layer
    NP = 128
    NS = BL // NP                 # slots per partition
    SH = BL + ZPAD                # shard rows in table (incl. zero rows)
    TBLR = n_cores * SH           # table rows
    CH = min(512, BL)             # node chunk for PSUM-bound ops
    NCH = BL // CH                # chunks per layer
    CB = CH // 128                # 128-blocks per chunk

    nc = bacc.Bacc("TRN2", target_bir_lowering=False, debug=False,
                   num_devices=n_cores)

    # ---- I/O ----
    xt = nc.dram_tensor("xt", [L, DA + DB, BL], F32, kind="ExternalInput").ap()
    w1t = nc.dram_tensor("w1t", [DA, G4], F32, kind="ExternalInput").ap()
    wbd = nc.dram_tensor("wbd", [DB, SROW], F32, kind="ExternalInput").ap()
    w34t = nc.dram_tensor("w34t", [SROW, G4], F32, kind="ExternalInput").ap()
    whht = nc.dram_tensor("whht", [H, G4], F32, kind="ExternalInput").ap()
    wfint = nc.dram_tensor("wfint", [H, 1], F32, kind="ExternalInput").ap()
    ebias = nc.dram_tensor("ebias", [SROW, 1], F32, kind="ExternalInput").ap()
    gbias = nc.dram_tensor("gbias", [G4, 1], F32, kind="ExternalInput").ap()
    bfin = nc.dram_tensor("bfin", [1, 1], F32, kind="ExternalInput").ap()
    NIDX = 2 * BL                 # gathered rows per layer
    IW = NIDX // 16               # idx tensor cols per layer
    goff = nc.dram_tensor("goff", [NP, L * IW], I16, kind="ExternalInput").ap()
    out = nc.dram_tensor("out", [1, BL], F32, kind="ExternalOutput").ap()
    dbg = None
    if os.environ.get("BASS_DEBUG_DUMP"):
        dbg = nc.dram_tensor("dbg", [L, NP, NS * SROW], F32,
                             kind="ExternalOutput").ap()

    # ---- internal DRAM ----
    aspace = "Shared" if n_cores > 4 else "Local"
    tbl = [nc.dram_tensor(f"tbl{i}", [TBLR, SPAD], F32,
                          addr_space=aspace).ap() for i in range(2)]
    cc_in = nc.dram_tensor("cc_in", [SH, SPAD], F32).ap()

    with tile.TileContext(nc) as tc:
        with (
            tc.tile_pool(name="const", bufs=1) as constp,
            tc.tile_pool(name="xin", bufs=2) as xinp,
            tc.tile_pool(name="emb", bufs=2) as embp,
            tc.tile_pool(name="gx", bufs=4) as gxp,
            tc.tile_pool(name="ph2", bufs=2) as ph2p,
            tc.tile_pool(name="zero", bufs=1) as zerop,
            tc.tile_pool(name="ps_emb", bufs=1, space="PSUM") as ps_emb,
            tc.tile_pool(name="ps_g", bufs=2, space="PSUM") as ps_g,
            tc.tile_pool(name="ps_h0t", bufs=1, space="PSUM") as ps_h0t,
            tc.tile_pool(name="ps_gh", bufs=2, space="PSUM") as ps_gh,
            tc.tile_pool(name="ps_nm", bufs=2, space="PSUM") as ps_nm,
        ):
            # ---- constants to SBUF ----
            w1t_s = constp.tile([DA, G4], F32)
            nc.sync.dma_start(w1t_s[:], w1t[:])
            wbd_s = constp.tile([DB, SROW], F32)
            nc.sync.dma_start(wbd_s[:], wbd[:])
            w34t_s = constp.tile([SROW, G4], F32)
            nc.sync.dma_start(w34t_s[:], w34t[:])
            whht_s = constp.tile([H, G4], F32)
            nc.sync.dma_start(whht_s[:], whht[:])
            wfint_s = constp.tile([H, 1], F32)
            nc.sync.dma_start(wfint_s[:], wfint[:])
            ebias_s = constp.tile([SROW, 1], F32)
            nc.sync.dma_start(ebias_s[:], ebias[:])
            gbias_s = constp.tile([G4, 1], F32)
            nc.sync.dma_start(gbias_s[:], gbias[:])
            bfin_s = constp.tile([1, 1], F32)
            nc.sync.dma_start(bfin_s[:], bfin[:])
            goff_s = constp.tile([NP, L * IW], I16)
            nc.sync.dma_start(goff_s[:], goff[:])
            ident = constp.tile([NP, NP], F32)
            make_identity(nc, ident[:])

            # persistent state tiles (double-buffered by step parity);
            # pad columns [SROW:SPAD] zeroed once and never rewritten
            state_a = constp.tile([NP, NS, SPAD], F32)
            nc.vector.memset(state_a[:], 0.0)
            state_b = constp.tile([NP, NS, SPAD], F32)
            nc.vector.memset(state_b[:], 0.0)
            state_tiles = [state_a, state_b]

            # ---- zero-init tbl0 and cc_in (incl. its permanent zero tail) ----
            zcols = (SH * SPAD) // NP
            zt = zerop.tile([NP, zcols], F32)
            nc.vector.memset(zt[:], 0.0)
            for p in range(n_cores):
                nc.sync.dma_start(tbl[0][p * SH:(p + 1) * SH, :], zt[:])
            nc.sync.dma_start(cc_in[:], zt[:])

            gx_tiles = {}
            state = None

            for t in range(L):
                lt = L - 1 - t
                # ============ phase 1: gates_x(lt) ============
                xa = xinp.tile([DA, BL], F32, tag="xa")
                nc.sync.dma_start(xa[:], xt[lt, 0:DA, :])
                xb = xinp.tile([DB, BL], F32, tag="xb")
                nc.sync.dma_start(xb[:], xt[lt, DA:DA + DB, :])

                gx = gxp.tile([G4, BL], F32)
                gx_tiles[t] = gx
                emb = embp.tile([SROW, BL], F32)
                for c in range(NCH):
                    sl = slice(c * CH, (c + 1) * CH)
                    e_ps = ps_emb.tile([SROW, CH], F32)
                    nc.tensor.matmul(e_ps[:], wbd_s[:], xb[:, sl],
                                     start=True, stop=True)
                    # relu(emb + eb) -> SBUF
                    nc.scalar.activation(emb[:, sl], e_ps[:], AF.Relu,
                                         bias=ebias_s[:, 0:1])
                    g_ps = ps_g.tile([G4, CH], F32)
                    nc.tensor.matmul(g_ps[:], w1t_s[:], xa[:, sl],
                                     start=True, stop=False)
                    nc.tensor.matmul(g_ps[:], w34t_s[:], emb[:, sl],
                                     start=False, stop=True)
                    # gates_x + bias -> SBUF ring
                    nc.vector.tensor_scalar(gx[:, sl], g_ps[:],
                                            gbias_s[:, 0:1], None, ALU.add)

                # ============ phase 2: scan step t (layer lt) ============
                # gather children state rows: pairs[p, s, k, :] from table.
                # Chunked to <=1024 idxs per call (SWDGE desc ring capacity).
                pairs = ph2p.tile([NP, NS, 2, SPAD], F32, tag="pairs")
                GC = min(1024, NIDX)          # idxs per gather call
                SC = GC // 256                # node slots per gather call
                for c4 in range(NIDX // GC):
                    nc.gpsimd.dma_gather(
                        pairs[:, c4 * SC:(c4 + 1) * SC, :, :].rearrange(
                            "p s two d -> p (s two) d"),
                        tbl[t % 2][:],
                        goff_s[:, t * IW + c4 * (GC // 16):
                               t * IW + (c4 + 1) * (GC // 16)],
                        GC, GC, SPAD,
                    )
                # h0c0 = child0 + child1  (node-major [128, NS, 40])
                h0c0 = ph2p.tile([NP, NS, SROW], F32, tag="h0c0")
                nc.vector.tensor_tensor(h0c0[:], pairs[:, :, 0, 0:SROW],
                                        pairs[:, :, 1, 0:SROW], ALU.add)

                # h0 -> feature-major via PE transposes; W_hh matmul; combine
                gcomb = ph2p.tile([G4, BL], F32, tag="gcomb")
                h0fm = ph2p.tile([H, BL], F32, tag="h0fm")
                for c in range(NCH):
                    sl = slice(c * CH, (c + 1) * CH)
                    h0t_ps = ps_h0t.tile([SROW, CH], F32)
                    for j in range(CB):
                        s = c * CB + j
                        nc.tensor.transpose(
                            h0t_ps[:, j * 128:(j + 1) * 128],
                            h0c0[:, s, :], ident[:])
                    # copy h rows to SBUF feature-major
                    nc.scalar.activation(h0fm[:, sl], h0t_ps[0:H, :], AF.Copy)
                    gh_ps = ps_gh.tile([G4, CH], F32, tag="gh")
                    nc.tensor.matmul(gh_ps[:], whht_s[:], h0fm[:, sl],
                                     start=True, stop=True)
                    nc.vector.tensor_tensor(gcomb[:, sl], gx_tiles[t][:, sl],
                                            gh_ps[:], ALU.add)
                del gx_tiles[t]

                # transpose gates to node-major; activations; pointwise
                state = state_tiles[t % 2]
                sig = ph2p.tile([NP, NS, 3 * H], F32, tag="sig")
                tng = ph2p.tile([NP, NS, H], F32, tag="tng")
                tnc = ph2p.tile([NP, NS, H], F32, tag="tnc")
                fha = ph2p.tile([NP, NS, H], F32, tag="fha")
                t1 = ph2p.tile([NP, NS, H], F32, tag="t1")
                for c in range(NCH):
                    nm_ps = ps_nm.tile([NP, CB, 128], F32)
                    for j in range(CB):
                        s = c * CB + j
                        nc.tensor.transpose(
                            nm_ps[:, j, 0:G4],
                            gcomb[:, s * 128:(s + 1) * 128],
                            ident[0:G4, 0:G4])
                    ssl = slice(c * CB, (c + 1) * CB)
                    # gate order (host-permuted): [i, f, o | g]
                    nc.scalar.activation(sig[:, ssl, :], nm_ps[:, :, 0:3 * H],
                                         AF.Sigmoid)
                    nc.scalar.activation(tng[:, ssl, :],
                                         nm_ps[:, :, 3 * H:G4], AF.Tanh)
                # c_new = 0.5*sigmoid(f)*c0 + sigmoid(i)*tanh(g)
                nc.vector.tensor_tensor(t1[:], sig[:, :, 0:H], tng[:], ALU.mult)
                nc.vector.tensor_scalar(fha[:], sig[:, :, H:2 * H], 0.5, None,
                                        ALU.mult)
                nc.vector.tensor_tensor(fha[:], fha[:], h0c0[:, :, H:SROW],
                                        ALU.mult)
                nc.vector.tensor_tensor(state[:, :, H:SROW], fha[:], t1[:],
                                        ALU.add)
                nc.scalar.activation(tnc[:], state[:, :, H:SROW], AF.Tanh)
                # h_new = sigmoid(o) * tanh(c_new)
                nc.vector.tensor_tensor(state[:, :, 0:H], sig[:, :, 2 * H:3 * H],
                                        tnc[:], ALU.mult)

                if dbg is not None:
                    nc.sync.dma_start(dbg[t], state[:, :, 0:SROW])
                if t < L - 1:
                    # exchange: shard -> cc_in -> AllGather -> tbl[(t+1)%2]
                    nc.sync.dma_start(cc_in[0:BL, :], state[:])
                    nc.gpsimd.collective_compute(
                        "AllGather",
                        ALU.bypass,
                        replica_groups=[list(range(n_cores))],
                        ins=[cc_in.opt()],
                        outs=[tbl[(t + 1) % 2].opt()],
                    )

            # ---- final projection: out = h @ wfin + bfin ----
            hfm = ph2p.tile([H, BL], F32, tag="h0fm")
            o_sb = ph2p.tile([1, BL], F32, tag="osb")
            for c in range(NCH):
                sl = slice(c * CH, (c + 1) * CH)
                h0t_ps = ps_h0t.tile([SROW, CH], F32)
                for j in range(CB):
                    s = c * CB + j
                    nc.tensor.transpose(h0t_ps[0:H, j * 128:(j + 1) * 128],
                                        state[:, s, 0:H], ident[:])
                nc.scalar.activation(hfm[:, sl], h0t_ps[0:H, :], AF.Copy)
                o_ps_full = ps_gh.tile([G4, CH], F32, tag="gh")
                o_ps = o_ps_full[0:1, :]
                nc.tensor.matmul(o_ps[:], wfint_s[:], hfm[:, sl],
                                 start=True, stop=True)
                nc.vector.tensor_scalar(o_sb[:, sl], o_ps[:],
                                        bfin_s[0:1, 0:1], None, ALU.add)
            nc.sync.dma_start(out[:], o_sb[:])

    nc.compile()
    return nc


def host_prepare(inputs, L, B, n_cores):
    """Host-side packing: transpose/concat features, fold constants, build
    gather offset tables. Returns (in_maps, shared_meta)."""
    BL = B // n_cores
    NP = 128
    NS = BL // NP
    SH = BL + ZPAD

    op = np.asarray(inputs["op_pad"], np.float32)
    at = np.asarray(inputs["attr_pad"], np.float32)
    fi = np.asarray(inputs["filter_pad"], np.float32)
    ou = np.asarray(inputs["output_pad"], np.float32)
    mp = np.asarray(inputs["mapping_pad"])
    Wf = np.asarray(inputs["Wf"], np.float32)
    bf = np.asarray(inputs["bf"], np.float32)
    Wo = np.asarray(inputs["Wo"], np.float32)
    bo = np.asarray(inputs["bo"], np.float32)
    W_ih = np.asarray(inputs["W_ih"], np.float32)
    W_hh = np.asarray(inputs["W_hh"], np.float32)
    b_ih = np.asarray(inputs["b_ih"], np.float32)
    b_hh = np.asarray(inputs["b_hh"], np.float32)
    W_fin = np.asarray(inputs["W_fin"], np.float32)
    b_fin = np.asarray(inputs["b_fin"], np.float32)

    # gate permutation i,f,g,o -> i,f,o,g
    perm = np.concatenate([np.arange(0, 2 * H), np.arange(3 * H, 4 * H),
                           np.arange(2 * H, 3 * H)])

    # feature-major concat [L, 222, B]
    X = np.concatenate([op, at, fi, ou], axis=2).transpose(0, 2, 1)
    X = np.ascontiguousarray(X)

    w1t = np.ascontiguousarray(W_ih[perm, :DA].T)                 # [96, 80]
    wbd = np.zeros((DB, SROW), np.float32)
    wbd[:D_FILT, :H] = Wf.T
    wbd[D_FILT:, H:] = Wo.T
    w34t = np.ascontiguousarray(W_ih[perm, DA:DA + SROW].T)       # [40, 80]
    whht = np.ascontiguousarray(0.5 * W_hh[perm, :].T)            # [20, 80]
    wfint = np.ascontiguousarray(W_fin.T)                         # [20, 1]
    ebias = np.concatenate([bf, bo]).reshape(SROW, 1)
    gbias = (b_ih + b_hh)[perm].reshape(G4, 1)
    bfin = b_fin.reshape(1, 1)

    # gather indices (dma_gather int16): scan step t processes layer
    # lt = L-1-t, gathering from state of layer lt+1 (written at step t-1;
    # step 0 reads zeros). Table row of global node g (owner core
    # p = g // BL, local j = g % BL):  p * SH + (j % 128) * NS + (j // 128)
    # zero row: BL (shard 0's zero tail). dma_gather idx layout: gathered
    # row i -> dst[i % 128, i // 128]; idx value at [i % 16, i // 16],
    # replicated across the 8 Q7 16-partition groups. We want dst slot
    # (p, sigma=2*s+k) = child k of local node s*128+p, i.e. i = sigma*128+p.
    NIDX = 2 * BL
    IW = NIDX // 16
    goff_all = np.zeros((n_cores, 16, L * IW), np.int16)
    for t in range(L):
        lt = L - 1 - t
        if t == 0:
            goff_all[:, :, 0:IW] = BL  # zero row of shard 0
            continue
        m = mp[lt]                                    # [B, 2] children
        mask = m != 0
        g = np.maximum(m - 1, 0)                      # [B, 2] global child
        p_own = g // BL
        j = g % BL
        row = p_own * SH + (j % NP) * NS + (j // NP)  # [B, 2]
        row = np.where(mask, row, BL)                 # masked -> zero row
        for c in range(n_cores):
            rl = row[c * BL:(c + 1) * BL]             # [BL, 2] (node i, child k)
            # flat gather index i = (2*(ilocal//128)+k)*128 + ilocal%128
            ilocal = np.arange(BL)
            flat = np.zeros(NIDX, np.int16)
            for k in range(2):
                i_ = (2 * (ilocal // NP) + k) * NP + (ilocal % NP)
                flat[i_] = rl[:, k]
            # idx value for gathered row i goes to [i % 16, i // 16]
            arr = flat.reshape(IW, 16).T              # [16, IW]
            goff_all[c, :, t * IW:(t + 1) * IW] = arr

    in_maps = []
    for c in range(n_cores):
        in_maps.append({
            "xt": np.ascontiguousarray(X[:, :, c * BL:(c + 1) * BL]),
            "w1t": w1t, "wbd": wbd, "w34t": w34t, "whht": whht,
            "wfint": wfint, "ebias": ebias, "gbias": gbias, "bfin": bfin,
            "goff": np.ascontiguousarray(np.tile(goff_all[c], (8, 1))),
        })
    return in_maps


_PROGRAM_CACHE = {}


def get_program(L, B, n_cores):
    key = (L, B, n_cores)
    if key not in _PROGRAM_CACHE:
        _PROGRAM_CACHE[key] = build_program(L, B, n_cores)
    return _PROGRAM_CACHE[key]


def kernel(**inputs):
    L = inputs["op_pad"].shape[0]
    B = inputs["op_pad"].shape[1]
    n_cores = 8
    BL = B // n_cores
    nc = get_program(L, B, n_cores)
    in_maps = host_prepare(inputs, L, B, n_cores)
    res = run_bass_kernel_spmd(nc, in_maps, core_ids=list(range(n_cores)),
                               trace=False)
    outs = [res.results[c]["out"].reshape(BL) for c in range(n_cores)]
    return np.concatenate(outs).reshape(B, 1).astype(np.float32)


if __name__ == "__main__":
    pass
